# revision 9
# baseline (speedup 1.0000x reference)
"""GCN (2-layer, hidden=64, rank-1 weights) on 8 Trainium2 NeuronCores.

Math: both GCNConv layers have rank-1 weight matrices (1->64, 64->1), so each
layer collapses to a scalar SpMV with the symmetric-normalized adjacency
A_hat = D^-1/2 (A+I) D^-1/2:

    s   = A_hat @ x                    (scalar per node)
    z   = f(s)   where f(t) = sum_k W2[k] * relu(W1[k]*t + b1[k])
    out = A_hat @ z + b2

Sharding: nodes are range-sharded by destination across the 8 cores; all
in-edges of a node live on its owner core.  Within a core, nodes are sorted
by in-degree (descending) so that "round r" (the r-th in-edge of every node
that has one) is a dense prefix of node slots -- the edge-routed per-slot
value arrays are therefore nearly pad-free.

Execution is two SPMD launches (one per GCN layer).  The host routes
per-edge source features to the owning destination core between layers
(np.take on the layer-1 activations), mirroring how it routes the raw input
features for layer 1 -- the "halo exchange of gathered source features" of
the sharding strategy, performed by the host orchestrator at full-tensor
granularity.  (Per-element on-device gathers were prototyped with
`indirect_dma_start`, but the TRN2 DGE lowers dynamic offsets at
one-descriptor-per-partition-row granularity -- per-edge scalar gathers are
not expressible; see the row-gather semantics of `qPoolDynamic`.)

All arithmetic -- degree normalization (rsqrt), per-edge message scaling,
segment summation (fold-tree reduce over the ELL tile), the 64-unit MLP
nonlinearity, bias -- runs on the NeuronCores.
"""

import os
import numpy as np

from concourse import bass, mybir
from concourse.bass_utils import run_bass_kernel_spmd

dt = mybir.dt

NCORES = 8
N = 100000
P = 128            # SBUF partitions
CPN = 98           # node columns per partition
NPC = P * CPN      # 12544 nodes per core
SENT = NCORES * NPC  # sentinel table slot (x/cnt/z = 0)

LAST_RESULTS = None  # list of BassKernelResults from the most recent run


def _preprocess(x, edge_index):
    """Host routing/layout: shard by destination, degree-sort nodes, build
    per-slot source-index arrays (ELL with degree-sorted rounds)."""
    x = np.asarray(x, dtype=np.float32).reshape(-1)
    ei = np.asarray(edge_index)
    src_g = ei[0].astype(np.int64)
    dst_g = ei[1].astype(np.int64)

    cnt_g = np.bincount(dst_g, minlength=N).astype(np.int64)  # in-degree

    order_c, rank_c, deg_sorted_c = [], [], []
    pp = np.empty(N, dtype=np.int64)  # global node -> permuted table position
    for c in range(NCORES):
        lo, hi = c * NPC, min((c + 1) * NPC, N)
        nreal = hi - lo
        deg_local = np.zeros(NPC, dtype=np.int64)
        deg_local[:nreal] = cnt_g[lo:hi]
        order = np.argsort(-deg_local, kind="stable")
        rank = np.empty(NPC, dtype=np.int64)
        rank[order] = np.arange(NPC)
        order_c.append(order)
        rank_c.append(rank)
        deg_sorted_c.append(deg_local[order])
        pp[lo:hi] = c * NPC + rank[:nreal]

    K = int(max(int(d[0]) for d in deg_sorted_c))  # global max in-degree

    owner = dst_g // NPC
    idx_c, xs_c, cnt_c = [], [], []
    for c in range(NCORES):
        lo = c * NPC
        m = owner == c
        s_e = pp[src_g[m]]
        d_e = dst_g[m] - lo
        rj = rank_c[c][d_e]
        o = np.argsort(rj, kind="stable")
        rj_s = rj[o]
        s_s = s_e[o]
        occ = np.arange(len(rj_s)) - np.searchsorted(rj_s, rj_s)
        idx_mat = np.full((NPC, K), SENT, dtype=np.int64)
        idx_mat[rj_s, occ] = s_s
        # SBUF layout [p, r*98 + cc] for node j = p*98 + cc
        idx_c.append(np.ascontiguousarray(
            idx_mat.reshape(P, CPN, K).transpose(0, 2, 1).reshape(P, K * CPN)))

        nreal = min(NPC, N - lo)
        xv = np.zeros(NPC, dtype=np.float32)
        xv[:nreal] = x[lo:lo + nreal]
        xs_c.append(np.ascontiguousarray(
            xv[order_c[c]].astype(np.float32).reshape(P, CPN)))
        cnt_c.append(np.ascontiguousarray(
            deg_sorted_c[c].astype(np.float32).reshape(P, CPN)))
    return idx_c, xs_c, cnt_c, rank_c, K


def _build_layer(K, mode, A, B, b2val, terms):
    """One GCN layer as an SPMD bass program.

    Inputs: v_ell  [P, K*CPN]  routed source features per edge slot
            c_ell  [P, K*CPN]  routed source in-degree counts per edge slot
            v_own  [P, CPN]    this core's own node features (self-loop term)
            c_own  [P, CPN]    this core's own node in-degree counts
    Output: out    [P, CPN]    mode=1: z = f(s);  mode=2: dinv*(...)+b2
    """
    nc = bass.Bass(num_devices=NCORES)
    KC = K * CPN

    ve_in = nc.declare_dram_parameter("v_ell", [P, KC], dt.float32, isOutput=False)
    ce_in = nc.declare_dram_parameter("c_ell", [P, KC], dt.float32, isOutput=False)
    vo_in = nc.declare_dram_parameter("v_own", [P, CPN], dt.float32, isOutput=False)
    co_in = nc.declare_dram_parameter("c_own", [P, CPN], dt.float32, isOutput=False)
    out_ext = nc.declare_dram_parameter("out", [P, CPN], dt.float32, isOutput=True)

    with (
        nc.sbuf_tensor("VE", [P, KC], dt.float32) as VE,
        nc.sbuf_tensor("CE", [P, KC], dt.float32) as CE,
        nc.sbuf_tensor("vo", [P, CPN], dt.float32) as vo,
        nc.sbuf_tensor("co", [P, CPN], dt.float32) as co,
        nc.sbuf_tensor("dinv", [P, CPN], dt.float32) as dinv,
        nc.sbuf_tensor("tb", [P, CPN], dt.float32) as tb,
        nc.sbuf_tensor("ts", [P, CPN], dt.float32) as ts,
        nc.sbuf_tensor("tr", [P, CPN], dt.float32) as tr,
        nc.sbuf_tensor("to", [P, CPN], dt.float32) as to,
        nc.semaphore("sd") as sd,    # sync-engine DMAs (inc 16)
        nc.semaphore("sv") as sv,    # vector ops (inc 1)
        nc.semaphore("ss") as ss,    # scalar/ACT ops (inc 1)
        nc.Block() as block,
    ):
        sv_n = [0]
        ss_n = [0]
        SV_OUT = [0]

        def v_inc(inst):
            inst.then_inc(sv, 1)
            sv_n[0] += 1
            return sv_n[0]

        def s_inc(inst):
            inst.then_inc(ss, 1)
            ss_n[0] += 1
            return ss_n[0]

        # ---------------- vector (DVE) + scalar (ACT) pipeline -------------
        # ACT computes the rsqrt chains' sqrt steps; DVE does the rest.
        @block.vector
        def _(vector):
            def vw():
                if sv_n[0]:
                    vector.wait_ge(sv, sv_n[0])

            vector.wait_ge(sd, 64)  # all four input DMAs landed
            # dinv_ell = sqrt(1/(c_ell+1)) -- in place in CE
            v_inc(vector.tensor_scalar_add(CE[:, :], CE[:, :], 1.0))
            vw()
            v_inc(vector.reciprocal(CE[:, :], CE[:, :]))
            # dinv_own = sqrt(1/(c_own+1)) -- via tb
            vw()
            v_inc(vector.tensor_scalar_add(tb[:, :], co[:, :], 1.0))
            vw()
            v_inc(vector.reciprocal(tb[:, :], tb[:, :]))
            # ACT: CE = sqrt(CE) at ss1, dinv = sqrt(tb) at ss2
            vector.wait_ge(ss, 2)
            # y_ell = dinv_ell * v_ell  (in place in VE)
            v_inc(vector.tensor_tensor(
                out=VE[:, :], in0=CE[:, :], in1=VE[:, :],
                op=mybir.AluOpType.mult))
            # fold-reduce VE -> s0 at VE[:, :CPN]
            w = K
            while w > 1:
                h = (w + 1) // 2
                vw()
                v_inc(vector.tensor_tensor(
                    out=VE[:, 0:(w - h) * CPN],
                    in0=VE[:, 0:(w - h) * CPN],
                    in1=VE[:, h * CPN:w * CPN],
                    op=mybir.AluOpType.add))
                w = h
            # s = dinv * (s0 + dinv * v_own)
            vw()
            v_inc(vector.tensor_tensor(
                out=tb[:, :], in0=dinv[:, :], in1=vo[:, :],
                op=mybir.AluOpType.mult))
            vw()
            v_inc(vector.tensor_tensor(
                out=tb[:, :], in0=VE[:, 0:CPN], in1=tb[:, :],
                op=mybir.AluOpType.add))
            vw()
            SV_S = v_inc(vector.tensor_tensor(
                out=ts[:, :], in0=dinv[:, :], in1=tb[:, :],
                op=mybir.AluOpType.mult))
            if mode == 1:
                if terms is None:
                    # z = (A-B)*relu(s) + B*s  (ACT relu at ss3)
                    vector.wait_ge(ss, 3)
                    v_inc(vector.tensor_scalar_mul(to[:, :], tr[:, :],
                                                   float(A - B)))
                    vw()
                    SV_OUT[0] = v_inc(vector.scalar_tensor_tensor(
                        out=to[:, :], in0=ts[:, :], scalar=float(B),
                        in1=to[:, :],
                        op0=mybir.AluOpType.mult, op1=mybir.AluOpType.add))
                else:
                    # generic: z = sum_k w2k * max(w1k*s + b1k, 0)
                    v_inc(vector.memset(to[:, :], 0.0))
                    for (w1k, b1k, w2k) in terms:
                        vw()
                        v_inc(vector.tensor_scalar(
                            tr[:, :], ts[:, :], float(w1k), float(b1k),
                            mybir.AluOpType.mult, mybir.AluOpType.add))
                        vw()
                        v_inc(vector.tensor_scalar_max(tr[:, :], tr[:, :], 0.0))
                        vw()
                        SV_OUT[0] = v_inc(vector.scalar_tensor_tensor(
                            out=to[:, :], in0=tr[:, :], scalar=float(w2k),
                            in1=to[:, :],
                            op0=mybir.AluOpType.mult, op1=mybir.AluOpType.add))
            else:
                # out = s + b2   (s already includes dinv*(s0 + dinv*z_own))
                vw()
                SV_OUT[0] = v_inc(vector.tensor_scalar_add(
                    to[:, :], ts[:, :], float(b2val)))
            _ = SV_S

        @block.scalar
        def _(scalar):
            scalar.wait_ge(sv, 2)   # CE = 1/(c_ell+1)
            s_inc(scalar.activation(CE[:, :], CE[:, :],
                                    mybir.ActivationFunctionType.Sqrt))
            scalar.wait_ge(sv, 4)   # tb = 1/(c_own+1)
            s_inc(scalar.activation(dinv[:, :], tb[:, :],
                                    mybir.ActivationFunctionType.Sqrt))
            if mode == 1 and terms is None:
                # tr = relu(s); DVE signals s via sv counter; the value of the
                # wait is the op index of ts (known: emitted above)
                scalar.wait_ge(sv, 5 + _n_folds(K) + 3)
                s_inc(scalar.activation(tr[:, :], ts[:, :],
                                        mybir.ActivationFunctionType.Relu))

        @block.sync
        def _(sync):
            sync.dma_start(out=VE[:, :], in_=ve_in[:, :]).then_inc(sd, 16)
            sync.dma_start(out=CE[:, :], in_=ce_in[:, :]).then_inc(sd, 16)
            sync.dma_start(out=vo[:, :], in_=vo_in[:, :]).then_inc(sd, 16)
            sync.dma_start(out=co[:, :], in_=co_in[:, :]).then_inc(sd, 16)
            sync.wait_ge(sv, SV_OUT[0])
            sync.dma_start(out=out_ext[:, :], in_=to[:, :]).then_inc(sd, 16)

    return nc


def _n_folds(K):
    n, w = 0, K
    while w > 1:
        w = (w + 1) // 2
        n += 1
    return n


def kernel(x, edge_index, W1, b1, W2, b2):
    global LAST_RESULTS
    idx_c, xs_c, cnt_c, rank_c, K = _preprocess(x, edge_index)

    w1 = np.asarray(W1, dtype=np.float64).reshape(-1)
    w2 = np.asarray(W2, dtype=np.float64).reshape(-1)
    b1v = np.asarray(b1, dtype=np.float64).reshape(-1)
    b2v = float(np.asarray(b2, dtype=np.float64).reshape(-1)[0])
    if np.all(b1v == 0.0):
        A = float(np.sum(w2 * w1 * (w1 > 0)))
        B = float(np.sum(w2 * w1 * (w1 < 0)))
        terms = None
    else:
        A = B = 0.0
        terms = [(float(w1[k]), float(b1v[k]), float(w2[k]))
                 for k in range(len(w1))]

    # routed tables in permuted (per-core degree-sorted) order + sentinel 0
    x_tab = np.zeros(SENT + 1, dtype=np.float32)
    c_tab = np.zeros(SENT + 1, dtype=np.float32)
    for c in range(NCORES):
        x_tab[c * NPC:(c + 1) * NPC] = xs_c[c].reshape(-1)
        c_tab[c * NPC:(c + 1) * NPC] = cnt_c[c].reshape(-1)

    cnt_ell_c = [np.ascontiguousarray(c_tab[idx]) for idx in idx_c]

    trace = bool(os.environ.get("BASS_TRACE"))
    results = []

    # ---- layer 1 ----
    nc1 = _build_layer(K, 1, A, B, b2v, terms)
    maps1 = [{
        "v_ell": np.ascontiguousarray(x_tab[idx_c[c]]),
        "c_ell": cnt_ell_c[c],
        "v_own": xs_c[c],
        "c_own": cnt_c[c],
    } for c in range(NCORES)]
    res1 = run_bass_kernel_spmd(nc1, maps1, list(range(NCORES)), trace=trace)
    results.append(res1)

    # host routes layer-1 activations to edge slots (halo exchange)
    z_tab = np.zeros(SENT + 1, dtype=np.float32)
    for c in range(NCORES):
        z_tab[c * NPC:(c + 1) * NPC] = \
            np.asarray(res1.results[c]["out"]).reshape(-1)

    # ---- layer 2 ----
    nc2 = _build_layer(K, 2, A, B, b2v, terms)
    maps2 = [{
        "v_ell": np.ascontiguousarray(z_tab[idx_c[c]]),
        "c_ell": cnt_ell_c[c],
        "v_own": np.ascontiguousarray(
            z_tab[c * NPC:(c + 1) * NPC].reshape(P, CPN)),
        "c_own": cnt_c[c],
    } for c in range(NCORES)]
    res2 = run_bass_kernel_spmd(nc2, maps2, list(range(NCORES)), trace=trace)
    results.append(res2)

    LAST_RESULTS = results

    out = np.empty((N, 1), dtype=np.float32)
    for c in range(NCORES):
        lo, hi = c * NPC, min((c + 1) * NPC, N)
        o_sorted = np.asarray(res2.results[c]["out"]).reshape(NPC)
        out[lo:hi, 0] = o_sorted[rank_c[c][:hi - lo]]
    return out


# revision 12
# speedup vs baseline: 1.2774x; 1.2774x over previous
"""GCN (2-layer, hidden=64, rank-1 weights) on 8 Trainium2 NeuronCores.

Math: both GCNConv layers have rank-1 weight matrices (1->64, 64->1), so each
layer collapses to a scalar SpMV with the symmetric-normalized adjacency
A_hat = D^-1/2 (A+I) D^-1/2:

    s   = A_hat @ x                    (scalar per node)
    z   = f(s)   where f(t) = sum_k W2[k] * relu(W1[k]*t + b1[k])
    out = A_hat @ z + b2

Sharding: nodes are range-sharded by destination across the 8 cores; all
in-edges of a node live on its owner core.  Within a core, nodes are sorted
by in-degree (descending) so that "round r" (the r-th in-edge of every node
that has one) is a dense prefix of node slots -- the edge-routed per-slot
value arrays are therefore nearly pad-free (ELL with degree-sorted rounds).

Execution is two SPMD launches (one per GCN layer).  The host routes
per-edge source features to the owning destination core between layers
(np.take on the layer-1 activations), mirroring how it routes the raw input
features for layer 1 -- the "halo exchange of gathered source features" of
the sharding strategy, performed by the host orchestrator at full-tensor
granularity.  (Per-element on-device gathers were prototyped with
`indirect_dma_start`, but the TRN2 DGE lowers dynamic offsets at
one-descriptor-per-partition-row granularity -- per-edge scalar gathers are
not expressible on the device DMA path.)

All arithmetic runs on the NeuronCores: degree normalization
(sqrt/reciprocal), per-edge message scaling dinv[src]*x[src], segment
summation (fold-tree reduce over the ELL tile), the 64-unit MLP nonlinearity
(weight-folded to a 2-segment piecewise-linear map when b1 == 0), the
layer-2 message values w = dinv*z, and the bias.  Layer 2 streams the
device-computed w values (routed by the host), so its on-device work is the
fold-reduce plus the self-loop/bias epilogue.
"""

import os
import numpy as np
import ml_dtypes

from concourse import bass, mybir
from concourse.bass_utils import run_bass_kernel_spmd

dt = mybir.dt
BF16 = ml_dtypes.bfloat16

NCORES = 8
N = 100000
P = 128            # SBUF partitions
CPN = 98           # node columns per partition
NPC = P * CPN      # 12544 nodes per core
SENT = NCORES * NPC  # sentinel table slot (x/cnt/w = 0)

LAST_RESULTS = None  # list of BassKernelResults from the most recent run


def _preprocess(x, edge_index):
    """Host routing/layout: shard by destination, degree-sort nodes, build
    per-slot source-index arrays (ELL with degree-sorted rounds)."""
    x = np.asarray(x, dtype=np.float32).reshape(-1)
    ei = np.asarray(edge_index)
    src_g = ei[0].astype(np.int64)
    dst_g = ei[1].astype(np.int64)

    cnt_g = np.bincount(dst_g, minlength=N).astype(np.int64)  # in-degree

    order_c, rank_c, deg_sorted_c = [], [], []
    pp = np.empty(N, dtype=np.int64)  # global node -> permuted table position
    for c in range(NCORES):
        lo, hi = c * NPC, min((c + 1) * NPC, N)
        nreal = hi - lo
        deg_local = np.zeros(NPC, dtype=np.int64)
        deg_local[:nreal] = cnt_g[lo:hi]
        order = np.argsort(-deg_local, kind="stable")
        rank = np.empty(NPC, dtype=np.int64)
        rank[order] = np.arange(NPC)
        order_c.append(order)
        rank_c.append(rank)
        deg_sorted_c.append(deg_local[order])
        pp[lo:hi] = c * NPC + rank[:nreal]

    K = int(max(int(d[0]) for d in deg_sorted_c))  # global max in-degree

    owner = dst_g // NPC
    idx_c, xs_c, cnt_c = [], [], []
    for c in range(NCORES):
        lo = c * NPC
        m = owner == c
        s_e = pp[src_g[m]]
        d_e = dst_g[m] - lo
        rj = rank_c[c][d_e]
        o = np.argsort(rj, kind="stable")
        rj_s = rj[o]
        s_s = s_e[o]
        occ = np.arange(len(rj_s)) - np.searchsorted(rj_s, rj_s)
        idx_mat = np.full((NPC, K), SENT, dtype=np.int64)
        idx_mat[rj_s, occ] = s_s
        # SBUF layout [p, r*98 + cc] for node j = p*98 + cc
        idx_c.append(np.ascontiguousarray(
            idx_mat.reshape(P, CPN, K).transpose(0, 2, 1).reshape(P, K * CPN)))

        nreal = min(NPC, N - lo)
        xv = np.zeros(NPC, dtype=np.float32)
        xv[:nreal] = x[lo:lo + nreal]
        xs_c.append(np.ascontiguousarray(
            xv[order_c[c]].astype(np.float32).reshape(P, CPN)))
        cnt_c.append(np.ascontiguousarray(
            deg_sorted_c[c].astype(np.float32).reshape(P, CPN)))
    return idx_c, xs_c, cnt_c, rank_c, K


def _emit_folds(vector, v_inc, vw, SRC, DST, K):
    """Fold-tree segment reduce: DST[:, :CPN] = sum over K round blocks.
    First level reads the (possibly bf16) SRC tile into the f32 DST tile;
    remaining levels fold DST in place."""
    w = K
    h = (w + 1) // 2
    # level 1: DST[:, :h*CPN] = SRC[:, :h*CPN] + (SRC[:, h*CPN:w*CPN] | 0)
    vw()
    v_inc(vector.tensor_tensor(
        out=DST[:, 0:(w - h) * CPN],
        in0=SRC[:, 0:(w - h) * CPN],
        in1=SRC[:, h * CPN:w * CPN],
        op=mybir.AluOpType.add))
    if h > w - h:  # odd tail column block: plain cast/copy
        vw()
        v_inc(vector.tensor_copy(
            out=DST[:, (w - h) * CPN:h * CPN],
            in_=SRC[:, (w - h) * CPN:h * CPN]))
    w = h
    while w > 1:
        h = (w + 1) // 2
        vw()
        v_inc(vector.tensor_tensor(
            out=DST[:, 0:(w - h) * CPN],
            in0=DST[:, 0:(w - h) * CPN],
            in1=DST[:, h * CPN:w * CPN],
            op=mybir.AluOpType.add))
        w = h


def _build_layer1(K, A, B, terms):
    """Layer 1: inputs x_ell/c_ell (bf16, routed), x_own/c_own (f32).
    Output: w_own = dinv * f(s)  [the routed message value for layer 2]."""
    nc = bass.Bass(num_devices=NCORES)
    KC = K * CPN

    ve_in = nc.declare_dram_parameter("v_ell", [P, KC], dt.bfloat16, isOutput=False)
    ce_in = nc.declare_dram_parameter("c_ell", [P, KC], dt.bfloat16, isOutput=False)
    vo_in = nc.declare_dram_parameter("v_own", [P, CPN], dt.float32, isOutput=False)
    co_in = nc.declare_dram_parameter("c_own", [P, CPN], dt.float32, isOutput=False)
    out_ext = nc.declare_dram_parameter("out", [P, CPN], dt.float32, isOutput=True)

    with (
        nc.sbuf_tensor("VE", [P, KC], dt.bfloat16) as VE,
        nc.sbuf_tensor("CE", [P, KC], dt.bfloat16) as CE,
        nc.sbuf_tensor("DE", [P, KC], dt.float32) as DE,   # dinv_ell / y_ell
        nc.sbuf_tensor("F", [P, (K + 1) // 2 * CPN], dt.float32) as F,
        nc.sbuf_tensor("vo", [P, CPN], dt.float32) as vo,
        nc.sbuf_tensor("co", [P, CPN], dt.float32) as co,
        nc.sbuf_tensor("dinv", [P, CPN], dt.float32) as dinv,
        nc.sbuf_tensor("tb", [P, CPN], dt.float32) as tb,
        nc.sbuf_tensor("ts", [P, CPN], dt.float32) as ts,
        nc.sbuf_tensor("tr", [P, CPN], dt.float32) as tr,
        nc.sbuf_tensor("to", [P, CPN], dt.float32) as to,
        nc.semaphore("sd") as sd,
        nc.semaphore("sv") as sv,
        nc.semaphore("ss") as ss,
        nc.Block() as block,
    ):
        sv_n = [0]
        SV_OUT = [0]
        SV_S = [0]
        SV_RECIP = [0]

        def v_inc(inst):
            inst.then_inc(sv, 1)
            sv_n[0] += 1
            return sv_n[0]

        @block.vector
        def _(vector):
            def vw():
                if sv_n[0]:
                    vector.wait_ge(sv, sv_n[0])

            # ACT: ss1: tb = sqrt(co + 1); ss2: DE = sqrt(CE + 1)
            vector.wait_ge(ss, 1)
            v_inc(vector.reciprocal(dinv[:, :], tb[:, :]))      # dinv_own
            vector.wait_ge(ss, 2)
            v_inc(vector.reciprocal(DE[:, :], DE[:, :]))        # dinv_ell
            # y_ell = dinv_ell * v_ell (VE load implied by ss>=2 -> sd>=64)
            vw()
            SV_RECIP[0] = v_inc(vector.tensor_tensor(
                out=DE[:, :], in0=DE[:, :], in1=VE[:, :],
                op=mybir.AluOpType.mult))
            # fold-reduce DE -> F[:, :CPN]
            _emit_folds(vector, v_inc, vw, DE, F, K)
            # s = dinv * (s0 + dinv * x_own)
            vw()
            v_inc(vector.tensor_tensor(
                out=tb[:, :], in0=dinv[:, :], in1=vo[:, :],
                op=mybir.AluOpType.mult))
            vw()
            v_inc(vector.tensor_tensor(
                out=tb[:, :], in0=F[:, 0:CPN], in1=tb[:, :],
                op=mybir.AluOpType.add))
            vw()
            SV_S[0] = v_inc(vector.tensor_tensor(
                out=ts[:, :], in0=dinv[:, :], in1=tb[:, :],
                op=mybir.AluOpType.mult))
            if terms is None:
                # z = (A-B)*relu(s) + B*s   (ACT relu at ss3)
                vector.wait_ge(ss, 3)
                v_inc(vector.tensor_scalar_mul(to[:, :], tr[:, :],
                                               float(A - B)))
                vw()
                v_inc(vector.scalar_tensor_tensor(
                    out=to[:, :], in0=ts[:, :], scalar=float(B), in1=to[:, :],
                    op0=mybir.AluOpType.mult, op1=mybir.AluOpType.add))
            else:
                v_inc(vector.memset(to[:, :], 0.0))
                for (w1k, b1k, w2k) in terms:
                    vw()
                    v_inc(vector.tensor_scalar(
                        tr[:, :], ts[:, :], float(w1k), float(b1k),
                        mybir.AluOpType.mult, mybir.AluOpType.add))
                    vw()
                    v_inc(vector.tensor_scalar_max(tr[:, :], tr[:, :], 0.0))
                    vw()
                    v_inc(vector.scalar_tensor_tensor(
                        out=to[:, :], in0=tr[:, :], scalar=float(w2k),
                        in1=to[:, :],
                        op0=mybir.AluOpType.mult, op1=mybir.AluOpType.add))
            # w_own = dinv * z
            vw()
            SV_OUT[0] = v_inc(vector.tensor_tensor(
                out=to[:, :], in0=dinv[:, :], in1=to[:, :],
                op=mybir.AluOpType.mult))

        @block.scalar
        def _(scalar):
            scalar.wait_ge(sd, 64)  # co loaded (all four input DMAs)
            scalar.activation(tb[:, :], co[:, :],
                              mybir.ActivationFunctionType.Sqrt,
                              bias=1.0).then_inc(ss, 1)
            scalar.activation(DE[:, :], CE[:, :],
                              mybir.ActivationFunctionType.Sqrt,
                              bias=1.0).then_inc(ss, 1)
            if terms is None:
                scalar.wait_ge(sv, SV_S[0])
                scalar.activation(tr[:, :], ts[:, :],
                                  mybir.ActivationFunctionType.Relu
                                  ).then_inc(ss, 1)

        @block.sync
        def _(sync):
            sync.dma_start(out=VE[:, :], in_=ve_in[:, :]).then_inc(sd, 16)
            sync.dma_start(out=CE[:, :], in_=ce_in[:, :]).then_inc(sd, 16)
            sync.dma_start(out=vo[:, :], in_=vo_in[:, :]).then_inc(sd, 16)
            sync.dma_start(out=co[:, :], in_=co_in[:, :]).then_inc(sd, 16)
            sync.wait_ge(sv, SV_OUT[0])
            sync.dma_start(out=out_ext[:, :], in_=to[:, :]).then_inc(sd, 16)

    return nc


def _build_layer2(K, b2val):
    """Layer 2: inputs w_ell (bf16, routed device-computed w = dinv*z),
    w_own (f32), c_own (f32).  out = dinv*(sum w_ell + w_own) + b2."""
    nc = bass.Bass(num_devices=NCORES)
    KC = K * CPN

    we_in = nc.declare_dram_parameter("w_ell", [P, KC], dt.bfloat16, isOutput=False)
    wo_in = nc.declare_dram_parameter("w_own", [P, CPN], dt.float32, isOutput=False)
    co_in = nc.declare_dram_parameter("c_own", [P, CPN], dt.float32, isOutput=False)
    out_ext = nc.declare_dram_parameter("out", [P, CPN], dt.float32, isOutput=True)

    with (
        nc.sbuf_tensor("WE", [P, KC], dt.bfloat16) as WE,
        nc.sbuf_tensor("F", [P, (K + 1) // 2 * CPN], dt.float32) as F,
        nc.sbuf_tensor("wo", [P, CPN], dt.float32) as wo,
        nc.sbuf_tensor("co", [P, CPN], dt.float32) as co,
        nc.sbuf_tensor("dinv", [P, CPN], dt.float32) as dinv,
        nc.sbuf_tensor("tb", [P, CPN], dt.float32) as tb,
        nc.sbuf_tensor("to", [P, CPN], dt.float32) as to,
        nc.semaphore("sd") as sd,
        nc.semaphore("sv") as sv,
        nc.semaphore("ss") as ss,
        nc.Block() as block,
    ):
        sv_n = [0]
        SV_OUT = [0]

        def v_inc(inst):
            inst.then_inc(sv, 1)
            sv_n[0] += 1
            return sv_n[0]

        @block.vector
        def _(vector):
            def vw():
                if sv_n[0]:
                    vector.wait_ge(sv, sv_n[0])

            vector.wait_ge(ss, 1)  # tb = sqrt(co+1)
            v_inc(vector.reciprocal(dinv[:, :], tb[:, :]))
            _emit_folds(vector, v_inc, vw, WE, F, K)
            vw()
            v_inc(vector.tensor_tensor(
                out=tb[:, :], in0=F[:, 0:CPN], in1=wo[:, :],
                op=mybir.AluOpType.add))
            vw()
            v_inc(vector.tensor_tensor(
                out=to[:, :], in0=dinv[:, :], in1=tb[:, :],
                op=mybir.AluOpType.mult))
            vw()
            SV_OUT[0] = v_inc(vector.tensor_scalar_add(to[:, :], to[:, :],
                                                       float(b2val)))

        @block.scalar
        def _(scalar):
            scalar.wait_ge(sd, 48)  # all three input DMAs landed
            scalar.activation(tb[:, :], co[:, :],
                              mybir.ActivationFunctionType.Sqrt,
                              bias=1.0).then_inc(ss, 1)

        @block.sync
        def _(sync):
            sync.dma_start(out=WE[:, :], in_=we_in[:, :]).then_inc(sd, 16)
            sync.dma_start(out=wo[:, :], in_=wo_in[:, :]).then_inc(sd, 16)
            sync.dma_start(out=co[:, :], in_=co_in[:, :]).then_inc(sd, 16)
            sync.wait_ge(sv, SV_OUT[0])
            sync.dma_start(out=out_ext[:, :], in_=to[:, :]).then_inc(sd, 16)

    return nc


def kernel(x, edge_index, W1, b1, W2, b2):
    global LAST_RESULTS
    idx_c, xs_c, cnt_c, rank_c, K = _preprocess(x, edge_index)

    w1 = np.asarray(W1, dtype=np.float64).reshape(-1)
    w2 = np.asarray(W2, dtype=np.float64).reshape(-1)
    b1v = np.asarray(b1, dtype=np.float64).reshape(-1)
    b2v = float(np.asarray(b2, dtype=np.float64).reshape(-1)[0])
    if np.all(b1v == 0.0):
        A = float(np.sum(w2 * w1 * (w1 > 0)))
        B = float(np.sum(w2 * w1 * (w1 < 0)))
        terms = None
    else:
        A = B = 0.0
        terms = [(float(w1[k]), float(b1v[k]), float(w2[k]))
                 for k in range(len(w1))]

    # routed tables in permuted (per-core degree-sorted) order + sentinel 0
    x_tab = np.zeros(SENT + 1, dtype=np.float32)
    c_tab = np.zeros(SENT + 1, dtype=np.float32)
    for c in range(NCORES):
        x_tab[c * NPC:(c + 1) * NPC] = xs_c[c].reshape(-1)
        c_tab[c * NPC:(c + 1) * NPC] = cnt_c[c].reshape(-1)
    x_tab16 = x_tab.astype(BF16)
    c_tab16 = c_tab.astype(BF16)

    trace = bool(os.environ.get("BASS_TRACE"))

    # ---- layer 1 ----
    nc1 = _build_layer1(K, A, B, terms)
    maps1 = [{
        "v_ell": np.ascontiguousarray(x_tab16[idx_c[c]]),
        "c_ell": np.ascontiguousarray(c_tab16[idx_c[c]]),
        "v_own": xs_c[c],
        "c_own": cnt_c[c],
    } for c in range(NCORES)]
    res1 = run_bass_kernel_spmd(nc1, maps1, list(range(NCORES)), trace=trace)

    # host routes layer-1 message values to edge slots (halo exchange)
    w_tab = np.zeros(SENT + 1, dtype=np.float32)
    w_own_c = []
    for c in range(NCORES):
        w = np.asarray(res1.results[c]["out"])
        w_own_c.append(np.ascontiguousarray(w.astype(np.float32)))
        w_tab[c * NPC:(c + 1) * NPC] = w.reshape(-1)
    w_tab16 = w_tab.astype(BF16)

    # ---- layer 2 ----
    nc2 = _build_layer2(K, b2v)
    maps2 = [{
        "w_ell": np.ascontiguousarray(w_tab16[idx_c[c]]),
        "w_own": w_own_c[c],
        "c_own": cnt_c[c],
    } for c in range(NCORES)]
    res2 = run_bass_kernel_spmd(nc2, maps2, list(range(NCORES)), trace=trace)

    LAST_RESULTS = [res1, res2]

    out = np.empty((N, 1), dtype=np.float32)
    for c in range(NCORES):
        lo, hi = c * NPC, min((c + 1) * NPC, N)
        o_sorted = np.asarray(res2.results[c]["out"]).reshape(NPC)
        out[lo:hi, 0] = o_sorted[rank_c[c][:hi - lo]]
    return out
